# revision 49
# baseline (speedup 1.0000x reference)
"""Causal attention + output projection on 8 Trainium2 NeuronCores, v6.

Problem (hardcoded): B=2, H=12, T=2048, D=64, DIM=768, fp32.

Sharding: 24 (b, h) pairs -> 3 heads per core; cores 0-3 take b=0,
cores 4-7 take b=1.  Each core computes attention for its 3 heads plus
the partial output projection; the host sums the 4 bf16 partials per
batch in fp32.  No collectives.

All matmuls run in bf16.  fp8 variants (DoubleRow at 0.5 cyc/row) were
measured to be off the error budget: independent per-element noise eps
on softmax weights or logits lands at ~sqrt(2)*eps relative-to-absmax
on the output (the 1/sqrt(Neff) attenuation cancels against the
output's own scale), so fp8's ~1.8% rms quantization alone costs
~2e-2.  The additive attention bias is applied MULTIPLICATIVELY:
the host ships eb = exp(bias) (0 where causally masked) in bf16 and
the device computes P = exp(qk) * eb -- exact masking for free.

Engine budget per core, measured (Act is the wall; exp is
scalar-engine-only at 0.833 ns/col + 185 ns SBUF-access per instr):
  Act  57.9us  exp (60 instrs) + 4 tail proj copies
  PE   54.1us  QK 21.8 + PV 21.8 + proj 10.2 + overheads
  DVE  ~48us   eb-mul share (2x bf16 mode) + reciprocal + normalize
               + PSUM->SBUF proj copies (384-wide)
  Pool ~42us   eb-mul share (chunks 2-3 only) + eb/va/out DMA (SWDGE)
  SP   ~44us   eb/qk/w/out DMA
Modeled span 69537 ns (vs 83772 baseline).

Scheduling (the engine queues are IN-ORDER, so emission order is the
schedule): three head streams round-robin per region (head-major for
the first/last chunk); PV lags 1-3 regions; psl is a 2-deep [128,1024]
PSUM ring; psy is one bank per head, allocated lazily at first use.
Chunk j's final-region PV flush + normalizations and its projection
blocks + out DMAs are deferred into chunk j+1's emission stream and
drained ONE per (r,h) iteration -- any burst of deferred work
head-of-line-blocks the in-order PE queue on a DVE copy and starves
Act.  Mid-stream projection PSUM ping-pongs on the single psp bank
(no psy-tag reuse -> no cross-chunk buffer-ordering deadlocks); only
the last chunk's burst rotates all 4 spare banks and splits its copies
Act/DVE.  Startup DMAs are ordered by first-use deadline with the eb
stream mostly on SP; Pool's transfers all land before chunk 2 so
mid-stream Pool eb-muls never queue behind a DMA.
"""

import math

import numpy as np
import ml_dtypes

B, H, T, D = 2, 12, 2048, 64
DIM = H * D
NCORES = 8
HPC = 3            # heads per core
P = 128
QC = 512           # q-chunk width
NJ = T // QC       # 4 q-chunks
NT = T // P        # 16 s-tiles

BF = ml_dtypes.bfloat16

_PROGRAM = None


def _c0(i, j):
    return max(0, P * i - QC * j)


def _regions(j):
    """[(r, wa, wb)] for chunk j: region r covers s-tiles 2r, 2r+1."""
    out = []
    for r in range(2 * (j + 1)):
        wa = QC - _c0(2 * r, j)
        wb = QC - _c0(2 * r + 1, j)
        out.append((r, wa, wb))
    return out


# eb packed offsets, (j, h)-major: chunk j's blocks for all heads are
# contiguous; within (j, h) the region blocks are r-major, each block
# [128 part, wa+wb] bf16 (tile-a cols then tile-b cols, causally trimmed).
EB_JCOLS = [sum(wa + wb for _, wa, wb in _regions(_j)) for _j in range(NJ)]
EB_JOFF = [0]
for _j in range(NJ):
    EB_JOFF.append(EB_JOFF[-1] + HPC * EB_JCOLS[_j])
EB_TOT = EB_JOFF[-1]   # 52224 cols per core


def _eb_off(j, h):
    return EB_JOFF[j] + h * EB_JCOLS[j]


def _build_program():
    import concourse.bass as bass
    import concourse.mybir as mybir
    import concourse.tile as tile
    from concourse import bacc
    from contextlib import ExitStack

    dt = mybir.dt
    f32 = dt.float32
    bf16 = dt.bfloat16
    EXP = mybir.ActivationFunctionType.Exp
    ds = bass.ds

    nc = bacc.Bacc("TRN2", num_devices=NCORES)
    k16_d = nc.declare_dram_parameter("k16", [D, HPC * T], bf16, isOutput=False)
    q16_d = nc.declare_dram_parameter("q16", [D, HPC * T], bf16, isOutput=False)
    eb_d = nc.declare_dram_parameter("eb", [P, EB_TOT], bf16,
                                     isOutput=False)
    va_d = nc.declare_dram_parameter("va", [P, HPC * NT * P], bf16,
                                     isOutput=False)
    w01_d = nc.declare_dram_parameter("w01", [P, DIM], bf16, isOutput=False)
    w2_d = nc.declare_dram_parameter("w2", [D, DIM], bf16, isOutput=False)
    out_d = nc.declare_dram_parameter("out", [T, DIM], bf16, isOutput=True)

    OB = 384  # projection o-block width (two blocks per 768-wide tb)

    with tile.TileContext(nc) as tc, ExitStack() as ctx:
        const_pool = ctx.enter_context(tc.tile_pool(name="const", bufs=1))
        k16_t = const_pool.tile([D, HPC * T], bf16)
        q16_t = const_pool.tile([D, HPC * T], bf16)
        eb_t = const_pool.tile([P, EB_TOT], bf16)
        va_t = const_pool.tile([P, HPC * NT * P], bf16)
        w01_t = const_pool.tile([P, DIM], bf16)
        w2_t = const_pool.tile([D, DIM], bf16)

        def eb_dma(eng, h, j, r0, r1):
            regs = _regions(j)
            o0 = _eb_off(j, h) + sum(wa + wb for _, wa, wb in regs[:r0])
            sz = sum(wa + wb for _, wa, wb in regs[r0:r1])
            eng.dma_start(eb_t[:, ds(o0, sz)], eb_d[:, ds(o0, sz)])

        # ---- startup DMAs, ordered by first-use deadline.  SP carries
        # most of the eb stream (it has no compute); Pool's DMAs all land
        # before chunk-2 so mid-stream Pool eb-muls never queue behind a
        # transfer.  Chunk 0/1 muls run on DVE only for the same reason.
        nc.sync.dma_start(k16_t[:, ds(0, QC)], k16_d[:, ds(0, QC)])
        nc.sync.dma_start(q16_t[:, ds(0, QC)], q16_d[:, ds(0, QC)])
        eb_dma(nc.gpsimd, 0, 0, 0, 2)
        nc.sync.dma_start(k16_t[:, ds(T, QC)], k16_d[:, ds(T, QC)])
        nc.sync.dma_start(q16_t[:, ds(T, QC)], q16_d[:, ds(T, QC)])
        eb_dma(nc.gpsimd, 1, 0, 0, 2)
        nc.sync.dma_start(k16_t[:, ds(2 * T, QC)], k16_d[:, ds(2 * T, QC)])
        nc.sync.dma_start(q16_t[:, ds(2 * T, QC)], q16_d[:, ds(2 * T, QC)])
        eb_dma(nc.gpsimd, 2, 0, 0, 2)
        nc.gpsimd.dma_start(va_t[:, ds(0, NT * P)], va_d[:, ds(0, NT * P)])
        # chunk-1 q for all heads before the k tails (their first use is
        # earlier); k tails cover s-tiles 4..15 which only chunk-1 r2+ needs
        nc.sync.dma_start(q16_t[:, ds(QC, QC)], q16_d[:, ds(QC, QC)])
        nc.sync.dma_start(q16_t[:, ds(T + QC, QC)], q16_d[:, ds(T + QC, QC)])
        nc.sync.dma_start(q16_t[:, ds(2 * T + QC, QC)],
                          q16_d[:, ds(2 * T + QC, QC)])
        nc.sync.dma_start(w01_t[:], w01_d[:])
        nc.sync.dma_start(w2_t[:], w2_d[:])
        nc.sync.dma_start(k16_t[:, ds(QC, T - QC)], k16_d[:, ds(QC, T - QC)])
        eb_dma(nc.sync, 0, 1, 0, 2)
        nc.gpsimd.dma_start(va_t[:, ds(NT * P, NT * P)],
                            va_d[:, ds(NT * P, NT * P)])
        nc.sync.dma_start(k16_t[:, ds(T + QC, T - QC)],
                          k16_d[:, ds(T + QC, T - QC)])
        eb_dma(nc.gpsimd, 2, 1, 0, 4)
        eb_dma(nc.sync, 0, 1, 2, 4)
        nc.sync.dma_start(k16_t[:, ds(2 * T + QC, T - QC)],
                          k16_d[:, ds(2 * T + QC, T - QC)])
        eb_dma(nc.sync, 1, 1, 0, 4)
        nc.gpsimd.dma_start(va_t[:, ds(2 * NT * P, NT * P)],
                            va_d[:, ds(2 * NT * P, NT * P)])
        eb_dma(nc.gpsimd, 0, 2, 0, 3)
        eb_dma(nc.gpsimd, 2, 2, 0, 6)
        nc.sync.dma_start(q16_t[:, ds(2 * QC, QC)], q16_d[:, ds(2 * QC, QC)])
        nc.sync.dma_start(q16_t[:, ds(T + 2 * QC, QC)],
                          q16_d[:, ds(T + 2 * QC, QC)])
        nc.sync.dma_start(q16_t[:, ds(2 * T + 2 * QC, QC)],
                          q16_d[:, ds(2 * T + 2 * QC, QC)])
        eb_dma(nc.sync, 0, 2, 3, 6)
        eb_dma(nc.sync, 1, 2, 0, 6)
        nc.sync.dma_start(q16_t[:, ds(3 * QC, QC)], q16_d[:, ds(3 * QC, QC)])
        nc.sync.dma_start(q16_t[:, ds(T + 3 * QC, QC)],
                          q16_d[:, ds(T + 3 * QC, QC)])
        nc.sync.dma_start(q16_t[:, ds(2 * T + 3 * QC, QC)],
                          q16_d[:, ds(2 * T + 3 * QC, QC)])
        eb_dma(nc.gpsimd, 0, 3, 0, 4)
        eb_dma(nc.sync, 0, 3, 4, 8)
        eb_dma(nc.sync, 1, 3, 0, 8)
        eb_dma(nc.sync, 2, 3, 0, 8)

        def k_ap(h, i):
            return k16_t[:, ds(h * T + i * P, P)]

        def q_ap(h, j, c0):
            return q16_t[:, ds(h * T + j * QC + c0, QC - c0)]

        def va_ap(h, i):
            return va_t[:, ds((h * NT + i) * P, P)]

        with (
            tc.tile_pool(name="pexp", bufs=8) as pexp_pool,
            tc.tile_pool(name="rec", bufs=2) as rec_pool,
            tc.tile_pool(name="yt01", bufs=2) as yt01_pool,
            tc.tile_pool(name="yt2", bufs=2) as yt2_pool,
            tc.tile_pool(name="obig", bufs=4) as obig_pool,
            tc.tile_pool(name="psl", bufs=2, space="PSUM") as psl_pool,
            tc.tile_pool(name="psy", bufs=1, space="PSUM") as psy_pool,
            tc.tile_pool(name="psp", bufs=1, space="PSUM") as psp_pool,
        ):
            defer_q = []   # deferred thunks (proj blocks, out/eb DMAs)

            def drain(n):
                for _ in range(min(n, len(defer_q))):
                    defer_q.pop(0)()

            COPY = mybir.ActivationFunctionType.Copy

            def make_blk(tb, ob, bank, yt01_c, yt2_c, o_c, act_copy=False):
                # proj PSUM rotates over psp + the h2/h1 psy banks (their
                # next-chunk PVs are the last to need the banks back; lazy
                # psy allocation keeps buffer-reuse deps acyclic with the
                # in-order engine queues).  The last chunk also uses h0.
                def blk():
                    if bank == 0:
                        psp_t = psp_pool.tile([P, OB], f32)
                    else:
                        psp_t = psy_pool.tile([P, OB], f32, name="psy",
                                              tag=["h2", "h1", "h0"][bank - 1])
                    nc.tensor.matmul(
                        psp_t[:],
                        lhsT=yt01_c[:, tb * P:(tb + 1) * P],
                        rhs=w01_t[:, ds(ob * OB, OB)],
                        start=True, stop=False,
                    )
                    nc.tensor.matmul(
                        psp_t[:],
                        lhsT=yt2_c[:, tb * P:(tb + 1) * P],
                        rhs=w2_t[:, ds(ob * OB, OB)],
                        start=False, stop=True,
                    )
                    if act_copy:
                        nc.scalar.activation(
                            o_c[:, ds(tb * DIM + ob * OB, OB)], psp_t[:], COPY
                        )
                    else:
                        nc.vector.tensor_copy(
                            o_c[:, ds(tb * DIM + ob * OB, OB)], psp_t[:]
                        )
                return blk

            def make_out_dma(eng, jj, tb, o_c, ob=None):
                def dma():
                    if ob is None:
                        eng.dma_start(
                            out_d[jj * QC + tb * P:jj * QC + (tb + 1) * P, :],
                            o_c[:, ds(tb * DIM, DIM)],
                        )
                    else:
                        eng.dma_start(
                            out_d[jj * QC + tb * P:jj * QC + (tb + 1) * P,
                                  ob * OB:(ob + 1) * OB],
                            o_c[:, ds(tb * DIM + ob * OB, OB)],
                        )
                return dma

            mul_toggle = [0]

            for j in range(NJ):
                regs = _regions(j)
                nreg = len(regs)
                eoffs = [_eb_off(j, h) for h in range(HPC)]
                psy_ts = [None] * HPC
                pvqs = [[] for _ in range(HPC)]

                def get_psy(h, psy_ts=psy_ts):
                    if psy_ts[h] is None:
                        psy_ts[h] = psy_pool.tile([P, QC], f32, name="psy",
                                                  tag=f"h{h}")
                    return psy_ts[h]

                def emit_pv(h, r, pe_t, wa, wb, nreg=nreg,
                            get_psy=get_psy):
                    for t in range(2):
                        i = 2 * r + t
                        pos = 0 if t == 0 else wa
                        w = wa if t == 0 else wb
                        nc.tensor.matmul(
                            get_psy(h)[:, QC - w:QC],
                            lhsT=va_ap(h, i),
                            rhs=pe_t[:, ds(pos, w)],
                            start=(r == 0 and t == 0),
                            stop=(r == nreg - 1 and t == 1),
                        )

                if j == 0 or j == NJ - 1:
                    iter_order = [(r, wa, wb, h) for h in range(HPC)
                                  for r, wa, wb in regs]
                else:
                    iter_order = [(r, wa, wb, h) for r, wa, wb in regs
                                  for h in range(HPC)]
                yt01_t = yt01_pool.tile([P, QC], bf16)
                yt2_t = yt2_pool.tile([D, QC], bf16)

                def make_tailnorm(h, yt01_c, yt2_c, pvq, psy_c, epv,
                                  split=False):
                    def tail():
                        while pvq:
                            epv(*pvq.pop(0))
                        rec_t = rec_pool.tile([D, QC], f32)
                        nc.vector.reciprocal(rec_t[:], psy_c(h)[D:2 * D, :])
                        if h == 0:
                            ydst = yt01_c[0:D, :]
                        elif h == 1:
                            ydst = yt01_c[D:2 * D, :]
                        else:
                            ydst = yt2_c[:]
                        if split:
                            for tb in range(4):
                                nc.vector.tensor_mul(
                                    ydst[:, tb * P:(tb + 1) * P],
                                    psy_c(h)[0:D, tb * P:(tb + 1) * P],
                                    rec_t[:, tb * P:(tb + 1) * P])
                        else:
                            nc.vector.tensor_mul(ydst, psy_c(h)[0:D, :],
                                                 rec_t[:])
                    return tail

                it = 0
                last = j == NJ - 1
                for r, wa, wb, h in iter_order:
                    if True:
                        po = wa + wb
                        psl_t = psl_pool.tile([P, 2 * QC], f32)
                        pe_t = pexp_pool.tile([P, 2 * QC], bf16)
                        nc.tensor.matmul(
                            psl_t[:, 0:wa],
                            lhsT=k_ap(h, 2 * r),
                            rhs=q_ap(h, j, QC - wa),
                            start=True, stop=True,
                        )
                        nc.tensor.matmul(
                            psl_t[:, wa:po],
                            lhsT=k_ap(h, 2 * r + 1),
                            rhs=q_ap(h, j, QC - wb),
                            start=True, stop=True,
                        )
                        nc.scalar.activation(
                            pe_t[:, 0:po], psl_t[:, 0:po], EXP
                        )
                        if j < 2:
                            meng = nc.vector
                        else:
                            meng = (nc.gpsimd if mul_toggle[0] % 5 < 3
                                    else nc.vector)
                            mul_toggle[0] += 1
                        meng.tensor_mul(
                            pe_t[:, 0:po],
                            pe_t[:, 0:po],
                            eb_t[:, ds(eoffs[h], po)],
                        )
                        eoffs[h] += po
                        pvqs[h].append((h, r, pe_t, wa, wb))
                        lag = 1 if j == 0 else (3 if last else 2)
                        if last and r >= nreg - 2:
                            while pvqs[h]:
                                emit_pv(*pvqs[h].pop(0))
                        elif len(pvqs[h]) > (lag if it < 6 else 1):
                            emit_pv(*pvqs[h].pop(0))
                        if last and r == nreg - 1:
                            # this head's stream is done; flush + norm now
                            # so only h2's norm remains after the last exp
                            defer_q.append(make_tailnorm(
                                h, yt01_t, yt2_t, pvqs[h], get_psy, emit_pv,
                                split=False))
                        if it >= 2:
                            drain(1)
                        it += 1

                if not last:
                    # tails go to the queue FRONT: they must drain before
                    # the next chunk's first lazy-psy PV allocation
                    for h in reversed(range(HPC)):
                        defer_q.insert(0, make_tailnorm(
                            h, yt01_t, yt2_t, pvqs[h], get_psy, emit_pv))

                o_t = obig_pool.tile([P, 4 * DIM], bf16)
                for tb in range(4):
                    for ob in range(2):
                        # mid-stream proj lives on psp only (no psy-tag
                        # ordering constraints); the last chunk's burst
                        # rotates all 4 banks and splits copies Act/DVE
                        defer_q.append(make_blk(
                            tb, ob, (tb * 2 + ob) % 4 if last else 0,
                            yt01_t, yt2_t, o_t,
                            act_copy=(last and (tb * 2 + ob) % 2 == 1)))
                        if last:
                            # per-half DMAs right after their copy so the
                            # final transfer is small and starts early
                            defer_q.append(make_out_dma(
                                nc.gpsimd if ob == 0 else nc.sync,
                                j, tb, o_t, ob=ob))
                    if not last:
                        defer_q.append(make_out_dma(
                            nc.sync if tb % 2 == 0 else nc.gpsimd,
                            j, tb, o_t))

            drain(len(defer_q))

    nc.finalize()
    return nc


def _get_program():
    global _PROGRAM
    if _PROGRAM is None:
        _PROGRAM = _build_program()
    return _PROGRAM


def make_in_maps(q, k, v, attn_bias, W_proj):
    """Host-side sharding/layout prep: one input map per core."""
    q = np.asarray(q, dtype=np.float32)
    k = np.asarray(k, dtype=np.float32)
    v = np.asarray(v, dtype=np.float32)
    attn_bias = np.asarray(attn_bias, dtype=np.float32)
    W_proj = np.asarray(W_proj, dtype=np.float32)

    scale = 1.0 / math.sqrt(D)
    w_heads = W_proj.reshape(H, D, DIM)
    smask = np.arange(T)[:, None] <= np.arange(T)[None, :]  # [s, q] valid

    in_maps = []
    for c in range(NCORES):
        bb = c // 4
        h0 = HPC * (c % 4)
        k16 = np.empty((HPC, D, T), dtype=BF)
        q16 = np.empty((HPC, D, T), dtype=BF)
        eb = np.empty((P, EB_TOT), dtype=BF)
        va = np.zeros((P, HPC, NT, P), dtype=np.float32)
        for h in range(HPC):
            hh = h0 + h
            k16[h] = k[bb, hh].T.astype(BF)
            q16[h] = (q[bb, hh].T * scale).astype(BF)
            ebf = np.exp(attn_bias[bb, hh].T) * smask  # [s, q]
            for j in range(NJ):
                off = _eb_off(j, h)
                for r, wa, wb in _regions(j):
                    eb[:, off:off + wa] = ebf[
                        2 * r * P:(2 * r + 1) * P,
                        (j + 1) * QC - wa:(j + 1) * QC].astype(BF)
                    off += wa
                    eb[:, off:off + wb] = ebf[
                        (2 * r + 1) * P:(2 * r + 2) * P,
                        (j + 1) * QC - wb:(j + 1) * QC].astype(BF)
                    off += wb
            va[:, h, :, 0:D] = (
                v[bb, hh].reshape(NT, P, D).transpose(1, 0, 2))
            va[:, h, :, D:] = 1.0
        # k16/q16: [d, (h, t)] with heads contiguous per partition
        in_maps.append(
            {
                "k16": k16.transpose(1, 0, 2).reshape(D, HPC * T),
                "q16": q16.transpose(1, 0, 2).reshape(D, HPC * T),
                "eb": eb,
                "va": va.reshape(P, HPC * NT * P).astype(BF),
                "w01": np.ascontiguousarray(
                    w_heads[h0:h0 + 2].reshape(P, DIM)).astype(BF),
                "w2": w_heads[h0 + 2].astype(BF),
                "out": np.zeros((T, DIM), dtype=BF),
            }
        )
    return in_maps


def assemble_output(results):
    """Sum the 4 per-core partial projections for each batch."""
    out = np.zeros((B, T, DIM), dtype=np.float32)
    for c in range(NCORES):
        out[c // 4] += np.asarray(results[c]["out"], dtype=np.float32)
    return out


def kernel(q, k, v, attn_bias, W_proj):
    from concourse.bass_utils import run_bass_kernel_spmd

    nc = _get_program()
    in_maps = make_in_maps(q, k, v, attn_bias, W_proj)
    res = run_bass_kernel_spmd(nc, in_maps, list(range(NCORES)))
    return assemble_output(res.results)


# revision 51
# speedup vs baseline: 1.0032x; 1.0032x over previous
"""Causal attention + output projection on 8 Trainium2 NeuronCores, v6.

Problem (hardcoded): B=2, H=12, T=2048, D=64, DIM=768, fp32.

Sharding: 24 (b, h) pairs -> 3 heads per core; cores 0-3 take b=0,
cores 4-7 take b=1.  Each core computes attention for its 3 heads plus
the partial output projection; the host sums the 4 bf16 partials per
batch in fp32.  No collectives.

All matmuls run in bf16.  fp8 variants (DoubleRow at 0.5 cyc/row) were
measured to be off the error budget: independent per-element noise eps
on softmax weights or logits lands at ~sqrt(2)*eps relative-to-absmax
on the output (the 1/sqrt(Neff) attenuation cancels against the
output's own scale), so fp8's ~1.8% rms quantization alone costs
~2e-2.  The additive attention bias is applied MULTIPLICATIVELY:
the host ships eb = exp(bias) (0 where causally masked) in bf16 and
the device computes P = exp(qk) * eb -- exact masking for free.

Engine budget per core, measured (Act is the wall; exp is
scalar-engine-only at 0.833 ns/col + 185 ns SBUF-access per instr):
  Act  57.9us  exp (60 instrs) + 4 tail proj copies
  PE   54.1us  QK 21.8 + PV 21.8 + proj 10.2 + overheads
  DVE  ~48us   eb-mul share (2x bf16 mode) + reciprocal + normalize
               + PSUM->SBUF proj copies (384-wide)
  Pool ~42us   eb-mul share (chunks 2-3 only) + eb/va/out DMA (SWDGE)
  SP   ~44us   eb/qk/w/out DMA
Modeled span 69537 ns (vs 83772 baseline).

Scheduling (the engine queues are IN-ORDER, so emission order is the
schedule): three head streams round-robin per region (head-major for
the first/last chunk); PV lags 1-3 regions; psl is a 2-deep [128,1024]
PSUM ring; psy is one bank per head, allocated lazily at first use.
Chunk j's final-region PV flush + normalizations and its projection
blocks + out DMAs are deferred into chunk j+1's emission stream and
drained ONE per (r,h) iteration -- any burst of deferred work
head-of-line-blocks the in-order PE queue on a DVE copy and starves
Act.  Mid-stream projection PSUM ping-pongs on the single psp bank
(no psy-tag reuse -> no cross-chunk buffer-ordering deadlocks); only
the last chunk's burst rotates all 4 spare banks and splits its copies
Act/DVE.  Startup DMAs are ordered by first-use deadline with the eb
stream mostly on SP; Pool's transfers all land before chunk 2 so
mid-stream Pool eb-muls never queue behind a DMA.
"""

import math

import numpy as np
import ml_dtypes

B, H, T, D = 2, 12, 2048, 64
DIM = H * D
NCORES = 8
HPC = 3            # heads per core
P = 128
QC = 512           # q-chunk width
NJ = T // QC       # 4 q-chunks
NT = T // P        # 16 s-tiles

BF = ml_dtypes.bfloat16

_PROGRAM = None


def _c0(i, j):
    return max(0, P * i - QC * j)


def _regions(j):
    """[(r, wa, wb)] for chunk j: region r covers s-tiles 2r, 2r+1."""
    out = []
    for r in range(2 * (j + 1)):
        wa = QC - _c0(2 * r, j)
        wb = QC - _c0(2 * r + 1, j)
        out.append((r, wa, wb))
    return out


# eb packed offsets, (j, h)-major: chunk j's blocks for all heads are
# contiguous; within (j, h) the region blocks are r-major, each block
# [128 part, wa+wb] bf16 (tile-a cols then tile-b cols, causally trimmed).
EB_JCOLS = [sum(wa + wb for _, wa, wb in _regions(_j)) for _j in range(NJ)]
EB_JOFF = [0]
for _j in range(NJ):
    EB_JOFF.append(EB_JOFF[-1] + HPC * EB_JCOLS[_j])
EB_TOT = EB_JOFF[-1]   # 52224 cols per core


def _eb_off(j, h):
    return EB_JOFF[j] + h * EB_JCOLS[j]


def _build_program():
    import concourse.bass as bass
    import concourse.mybir as mybir
    import concourse.tile as tile
    from concourse import bacc
    from contextlib import ExitStack

    dt = mybir.dt
    f32 = dt.float32
    bf16 = dt.bfloat16
    EXP = mybir.ActivationFunctionType.Exp
    ds = bass.ds

    nc = bacc.Bacc("TRN2", num_devices=NCORES)
    k16_d = nc.declare_dram_parameter("k16", [D, HPC * T], bf16, isOutput=False)
    q16_d = nc.declare_dram_parameter("q16", [D, HPC * T], bf16, isOutput=False)
    eb_d = nc.declare_dram_parameter("eb", [P, EB_TOT], bf16,
                                     isOutput=False)
    va_d = nc.declare_dram_parameter("va", [P, HPC * NT * P], bf16,
                                     isOutput=False)
    w01_d = nc.declare_dram_parameter("w01", [P, DIM], bf16, isOutput=False)
    w2_d = nc.declare_dram_parameter("w2", [D, DIM], bf16, isOutput=False)
    out_d = nc.declare_dram_parameter("out", [T, DIM], bf16, isOutput=True)

    OB = 384  # projection o-block width (two blocks per 768-wide tb)

    with tile.TileContext(nc) as tc, ExitStack() as ctx:
        const_pool = ctx.enter_context(tc.tile_pool(name="const", bufs=1))
        k16_t = const_pool.tile([D, HPC * T], bf16)
        q16_t = const_pool.tile([D, HPC * T], bf16)
        eb_t = const_pool.tile([P, EB_TOT], bf16)
        va_t = const_pool.tile([P, HPC * NT * P], bf16)
        w01_t = const_pool.tile([P, DIM], bf16)
        w2_t = const_pool.tile([D, DIM], bf16)

        def eb_dma(eng, h, j, r0, r1):
            regs = _regions(j)
            o0 = _eb_off(j, h) + sum(wa + wb for _, wa, wb in regs[:r0])
            sz = sum(wa + wb for _, wa, wb in regs[r0:r1])
            eng.dma_start(eb_t[:, ds(o0, sz)], eb_d[:, ds(o0, sz)])

        # ---- startup DMAs, ordered by first-use deadline.  SP carries
        # most of the eb stream (it has no compute); Pool's DMAs all land
        # before chunk-2 so mid-stream Pool eb-muls never queue behind a
        # transfer.  Chunk 0/1 muls run on DVE only for the same reason.
        nc.sync.dma_start(k16_t[:, ds(0, QC)], k16_d[:, ds(0, QC)])
        nc.sync.dma_start(q16_t[:, ds(0, QC)], q16_d[:, ds(0, QC)])
        eb_dma(nc.gpsimd, 0, 0, 0, 2)
        nc.sync.dma_start(k16_t[:, ds(T, QC)], k16_d[:, ds(T, QC)])
        nc.sync.dma_start(q16_t[:, ds(T, QC)], q16_d[:, ds(T, QC)])
        eb_dma(nc.gpsimd, 1, 0, 0, 2)
        nc.sync.dma_start(k16_t[:, ds(2 * T, QC)], k16_d[:, ds(2 * T, QC)])
        nc.sync.dma_start(q16_t[:, ds(2 * T, QC)], q16_d[:, ds(2 * T, QC)])
        eb_dma(nc.gpsimd, 2, 0, 0, 2)
        nc.gpsimd.dma_start(va_t[:, ds(0, NT * P)], va_d[:, ds(0, NT * P)])
        # chunk-1 q for all heads before the k tails (their first use is
        # earlier); k tails cover s-tiles 4..15 which only chunk-1 r2+ needs
        nc.sync.dma_start(q16_t[:, ds(QC, QC)], q16_d[:, ds(QC, QC)])
        nc.sync.dma_start(q16_t[:, ds(T + QC, QC)], q16_d[:, ds(T + QC, QC)])
        nc.sync.dma_start(q16_t[:, ds(2 * T + QC, QC)],
                          q16_d[:, ds(2 * T + QC, QC)])
        nc.sync.dma_start(w01_t[:], w01_d[:])
        nc.sync.dma_start(w2_t[:], w2_d[:])
        nc.sync.dma_start(k16_t[:, ds(QC, T - QC)], k16_d[:, ds(QC, T - QC)])
        eb_dma(nc.sync, 0, 1, 0, 2)
        nc.gpsimd.dma_start(va_t[:, ds(NT * P, NT * P)],
                            va_d[:, ds(NT * P, NT * P)])
        nc.sync.dma_start(k16_t[:, ds(T + QC, T - QC)],
                          k16_d[:, ds(T + QC, T - QC)])
        eb_dma(nc.gpsimd, 2, 1, 0, 4)
        eb_dma(nc.sync, 0, 1, 2, 4)
        nc.sync.dma_start(k16_t[:, ds(2 * T + QC, T - QC)],
                          k16_d[:, ds(2 * T + QC, T - QC)])
        eb_dma(nc.sync, 1, 1, 0, 4)
        nc.gpsimd.dma_start(va_t[:, ds(2 * NT * P, NT * P)],
                            va_d[:, ds(2 * NT * P, NT * P)])
        eb_dma(nc.gpsimd, 0, 2, 0, 3)
        eb_dma(nc.gpsimd, 2, 2, 0, 6)
        nc.sync.dma_start(q16_t[:, ds(2 * QC, QC)], q16_d[:, ds(2 * QC, QC)])
        nc.sync.dma_start(q16_t[:, ds(T + 2 * QC, QC)],
                          q16_d[:, ds(T + 2 * QC, QC)])
        nc.sync.dma_start(q16_t[:, ds(2 * T + 2 * QC, QC)],
                          q16_d[:, ds(2 * T + 2 * QC, QC)])
        eb_dma(nc.sync, 0, 2, 3, 6)
        eb_dma(nc.sync, 1, 2, 0, 6)
        nc.sync.dma_start(q16_t[:, ds(3 * QC, QC)], q16_d[:, ds(3 * QC, QC)])
        nc.sync.dma_start(q16_t[:, ds(T + 3 * QC, QC)],
                          q16_d[:, ds(T + 3 * QC, QC)])
        nc.sync.dma_start(q16_t[:, ds(2 * T + 3 * QC, QC)],
                          q16_d[:, ds(2 * T + 3 * QC, QC)])
        eb_dma(nc.gpsimd, 0, 3, 0, 4)
        eb_dma(nc.sync, 0, 3, 4, 8)
        eb_dma(nc.sync, 1, 3, 0, 8)
        eb_dma(nc.sync, 2, 3, 0, 8)

        def k_ap(h, i):
            return k16_t[:, ds(h * T + i * P, P)]

        def q_ap(h, j, c0):
            return q16_t[:, ds(h * T + j * QC + c0, QC - c0)]

        def va_ap(h, i):
            return va_t[:, ds((h * NT + i) * P, P)]

        with (
            tc.tile_pool(name="pexp", bufs=8) as pexp_pool,
            tc.tile_pool(name="rec", bufs=2) as rec_pool,
            tc.tile_pool(name="yt01", bufs=2) as yt01_pool,
            tc.tile_pool(name="yt2", bufs=2) as yt2_pool,
            tc.tile_pool(name="obig", bufs=4) as obig_pool,
            tc.tile_pool(name="psl", bufs=2, space="PSUM") as psl_pool,
            tc.tile_pool(name="psy", bufs=1, space="PSUM") as psy_pool,
            tc.tile_pool(name="psp", bufs=1, space="PSUM") as psp_pool,
        ):
            defer_q = []   # deferred thunks (proj blocks, out/eb DMAs)

            def drain(n):
                for _ in range(min(n, len(defer_q))):
                    defer_q.pop(0)()

            COPY = mybir.ActivationFunctionType.Copy

            def make_blk(tb, ob, bank, yt01_c, yt2_c, o_c, act_copy=False):
                # proj PSUM rotates over psp + the h2/h1 psy banks (their
                # next-chunk PVs are the last to need the banks back; lazy
                # psy allocation keeps buffer-reuse deps acyclic with the
                # in-order engine queues).  The last chunk also uses h0.
                def blk():
                    if bank == 0:
                        psp_t = psp_pool.tile([P, OB], f32)
                    else:
                        psp_t = psy_pool.tile([P, OB], f32, name="psy",
                                              tag=["h2", "h1", "h0"][bank - 1])
                    nc.tensor.matmul(
                        psp_t[:],
                        lhsT=yt01_c[:, tb * P:(tb + 1) * P],
                        rhs=w01_t[:, ds(ob * OB, OB)],
                        start=True, stop=False,
                    )
                    nc.tensor.matmul(
                        psp_t[:],
                        lhsT=yt2_c[:, tb * P:(tb + 1) * P],
                        rhs=w2_t[:, ds(ob * OB, OB)],
                        start=False, stop=True,
                    )
                    if act_copy:
                        nc.scalar.activation(
                            o_c[:, ds(tb * DIM + ob * OB, OB)], psp_t[:], COPY
                        )
                    else:
                        nc.vector.tensor_copy(
                            o_c[:, ds(tb * DIM + ob * OB, OB)], psp_t[:]
                        )
                return blk

            def make_out_dma(eng, jj, tb, o_c, ob=None):
                def dma():
                    if ob is None:
                        eng.dma_start(
                            out_d[jj * QC + tb * P:jj * QC + (tb + 1) * P, :],
                            o_c[:, ds(tb * DIM, DIM)],
                        )
                    else:
                        eng.dma_start(
                            out_d[jj * QC + tb * P:jj * QC + (tb + 1) * P,
                                  ob * OB:(ob + 1) * OB],
                            o_c[:, ds(tb * DIM + ob * OB, OB)],
                        )
                return dma

            mul_toggle = [0]

            for j in range(NJ):
                regs = _regions(j)
                nreg = len(regs)
                roffs = [0] * len(regs)
                for _r, _wa, _wb in regs[:-1]:
                    roffs[_r + 1] = roffs[_r] + _wa + _wb
                psy_ts = [None] * HPC
                pvqs = [[] for _ in range(HPC)]

                def get_psy(h, psy_ts=psy_ts):
                    if psy_ts[h] is None:
                        psy_ts[h] = psy_pool.tile([P, QC], f32, name="psy",
                                                  tag=f"h{h}")
                    return psy_ts[h]

                pv_done = [0] * HPC

                def emit_pv(h, r, pe_t, wa, wb, nreg=nreg,
                            get_psy=get_psy, pv_done=pv_done):
                    first = pv_done[h] == 0
                    pv_done[h] += 1
                    final = pv_done[h] == nreg
                    for t in range(2):
                        i = 2 * r + t
                        pos = 0 if t == 0 else wa
                        w = wa if t == 0 else wb
                        nc.tensor.matmul(
                            get_psy(h)[:, QC - w:QC],
                            lhsT=va_ap(h, i),
                            rhs=pe_t[:, ds(pos, w)],
                            start=(first and t == 0),
                            stop=(final and t == 1),
                        )

                if j == NJ - 1:
                    iter_order = [(r, wa, wb, h) for h in range(HPC)
                                  for r, wa, wb in reversed(regs)]
                elif j == 0:
                    iter_order = [(r, wa, wb, h) for h in range(HPC)
                                  for r, wa, wb in regs]
                else:
                    iter_order = [(r, wa, wb, h) for r, wa, wb in regs
                                  for h in range(HPC)]
                yt01_t = yt01_pool.tile([P, QC], bf16)
                yt2_t = yt2_pool.tile([D, QC], bf16)

                def make_tailnorm(h, yt01_c, yt2_c, pvq, psy_c, epv,
                                  split=False):
                    def tail():
                        while pvq:
                            epv(*pvq.pop(0))
                        rec_t = rec_pool.tile([D, QC], f32)
                        nc.vector.reciprocal(rec_t[:], psy_c(h)[D:2 * D, :])
                        if h == 0:
                            ydst = yt01_c[0:D, :]
                        elif h == 1:
                            ydst = yt01_c[D:2 * D, :]
                        else:
                            ydst = yt2_c[:]
                        if split:
                            for tb in range(4):
                                nc.vector.tensor_mul(
                                    ydst[:, tb * P:(tb + 1) * P],
                                    psy_c(h)[0:D, tb * P:(tb + 1) * P],
                                    rec_t[:, tb * P:(tb + 1) * P])
                        else:
                            nc.vector.tensor_mul(ydst, psy_c(h)[0:D, :],
                                                 rec_t[:])
                    return tail

                it = 0
                last = j == NJ - 1
                for r, wa, wb, h in iter_order:
                    if True:
                        po = wa + wb
                        psl_t = psl_pool.tile([P, 2 * QC], f32)
                        pe_t = pexp_pool.tile([P, 2 * QC], bf16)
                        nc.tensor.matmul(
                            psl_t[:, 0:wa],
                            lhsT=k_ap(h, 2 * r),
                            rhs=q_ap(h, j, QC - wa),
                            start=True, stop=True,
                        )
                        nc.tensor.matmul(
                            psl_t[:, wa:po],
                            lhsT=k_ap(h, 2 * r + 1),
                            rhs=q_ap(h, j, QC - wb),
                            start=True, stop=True,
                        )
                        nc.scalar.activation(
                            pe_t[:, 0:po], psl_t[:, 0:po], EXP
                        )
                        if j < 2:
                            meng = nc.vector
                        else:
                            meng = (nc.gpsimd if mul_toggle[0] % 5 < 3
                                    else nc.vector)
                            mul_toggle[0] += 1
                        meng.tensor_mul(
                            pe_t[:, 0:po],
                            pe_t[:, 0:po],
                            eb_t[:, ds(_eb_off(j, h) + roffs[r], po)],
                        )
                        pvqs[h].append((h, r, pe_t, wa, wb))
                        lag = 1 if j == 0 else (3 if last else 2)
                        hpos = it % nreg if last else r

                        def pop_pv(h=h):
                            # first PV per head must write the whole psy
                            # bank (a start=False matmul may not touch a
                            # mix of pending-zero and written bytes)
                            if pv_done[h] == 0:
                                for ii, e in enumerate(pvqs[h]):
                                    if e[3] == QC:
                                        return pvqs[h].pop(ii)
                            return pvqs[h].pop(0)

                        if last and hpos >= nreg - 2:
                            while pvqs[h]:
                                emit_pv(*pop_pv())
                        elif len(pvqs[h]) > (lag if it < 6 else 1):
                            emit_pv(*pop_pv())
                        if last and hpos == nreg - 1:
                            # this head's stream is done; flush + norm now
                            # so only h2's norm remains after the last exp
                            defer_q.append(make_tailnorm(
                                h, yt01_t, yt2_t, pvqs[h], get_psy, emit_pv,
                                split=False))
                        if it >= 2:
                            drain(1)
                        it += 1

                if not last:
                    # tails go to the queue FRONT: they must drain before
                    # the next chunk's first lazy-psy PV allocation
                    for h in reversed(range(HPC)):
                        defer_q.insert(0, make_tailnorm(
                            h, yt01_t, yt2_t, pvqs[h], get_psy, emit_pv))

                o_t = obig_pool.tile([P, 4 * DIM], bf16)
                for tb in range(4):
                    for ob in range(2):
                        # mid-stream proj lives on psp only (no psy-tag
                        # ordering constraints); the last chunk's burst
                        # rotates all 4 banks and splits copies Act/DVE
                        defer_q.append(make_blk(
                            tb, ob, (tb * 2 + ob) % 4 if last else 0,
                            yt01_t, yt2_t, o_t,
                            act_copy=(last and (tb * 2 + ob) % 2 == 1)))
                        if last:
                            # per-half DMAs right after their copy so the
                            # final transfer is small and starts early
                            defer_q.append(make_out_dma(
                                nc.gpsimd if ob == 0 else nc.sync,
                                j, tb, o_t, ob=ob))
                    if not last:
                        defer_q.append(make_out_dma(
                            nc.sync if tb % 2 == 0 else nc.gpsimd,
                            j, tb, o_t))

            drain(len(defer_q))

    nc.finalize()
    return nc


def _get_program():
    global _PROGRAM
    if _PROGRAM is None:
        _PROGRAM = _build_program()
    return _PROGRAM


def make_in_maps(q, k, v, attn_bias, W_proj):
    """Host-side sharding/layout prep: one input map per core."""
    q = np.asarray(q, dtype=np.float32)
    k = np.asarray(k, dtype=np.float32)
    v = np.asarray(v, dtype=np.float32)
    attn_bias = np.asarray(attn_bias, dtype=np.float32)
    W_proj = np.asarray(W_proj, dtype=np.float32)

    scale = 1.0 / math.sqrt(D)
    w_heads = W_proj.reshape(H, D, DIM)
    smask = np.arange(T)[:, None] <= np.arange(T)[None, :]  # [s, q] valid

    in_maps = []
    for c in range(NCORES):
        bb = c // 4
        h0 = HPC * (c % 4)
        k16 = np.empty((HPC, D, T), dtype=BF)
        q16 = np.empty((HPC, D, T), dtype=BF)
        eb = np.empty((P, EB_TOT), dtype=BF)
        va = np.zeros((P, HPC, NT, P), dtype=np.float32)
        for h in range(HPC):
            hh = h0 + h
            k16[h] = k[bb, hh].T.astype(BF)
            q16[h] = (q[bb, hh].T * scale).astype(BF)
            ebf = np.exp(attn_bias[bb, hh].T) * smask  # [s, q]
            for j in range(NJ):
                off = _eb_off(j, h)
                for r, wa, wb in _regions(j):
                    eb[:, off:off + wa] = ebf[
                        2 * r * P:(2 * r + 1) * P,
                        (j + 1) * QC - wa:(j + 1) * QC].astype(BF)
                    off += wa
                    eb[:, off:off + wb] = ebf[
                        (2 * r + 1) * P:(2 * r + 2) * P,
                        (j + 1) * QC - wb:(j + 1) * QC].astype(BF)
                    off += wb
            va[:, h, :, 0:D] = (
                v[bb, hh].reshape(NT, P, D).transpose(1, 0, 2))
            va[:, h, :, D:] = 1.0
        # k16/q16: [d, (h, t)] with heads contiguous per partition
        in_maps.append(
            {
                "k16": k16.transpose(1, 0, 2).reshape(D, HPC * T),
                "q16": q16.transpose(1, 0, 2).reshape(D, HPC * T),
                "eb": eb,
                "va": va.reshape(P, HPC * NT * P).astype(BF),
                "w01": np.ascontiguousarray(
                    w_heads[h0:h0 + 2].reshape(P, DIM)).astype(BF),
                "w2": w_heads[h0 + 2].astype(BF),
                "out": np.zeros((T, DIM), dtype=BF),
            }
        )
    return in_maps


def assemble_output(results):
    """Sum the 4 per-core partial projections for each batch."""
    out = np.zeros((B, T, DIM), dtype=np.float32)
    for c in range(NCORES):
        out[c // 4] += np.asarray(results[c]["out"], dtype=np.float32)
    return out


def kernel(q, k, v, attn_bias, W_proj):
    from concourse.bass_utils import run_bass_kernel_spmd

    nc = _get_program()
    in_maps = make_in_maps(q, k, v, attn_bias, W_proj)
    res = run_bass_kernel_spmd(nc, in_maps, list(range(NCORES)))
    return assemble_output(res.results)


# revision 53
# speedup vs baseline: 1.0107x; 1.0075x over previous
"""Causal attention + output projection on 8 Trainium2 NeuronCores, v6.

Problem (hardcoded): B=2, H=12, T=2048, D=64, DIM=768, fp32.

Sharding: 24 (b, h) pairs -> 3 heads per core; cores 0-3 take b=0,
cores 4-7 take b=1.  Each core computes attention for its 3 heads plus
the partial output projection; the host sums the 4 bf16 partials per
batch in fp32.  No collectives.

All matmuls run in bf16.  fp8 variants (DoubleRow at 0.5 cyc/row) were
measured to be off the error budget: independent per-element noise eps
on softmax weights or logits lands at ~sqrt(2)*eps relative-to-absmax
on the output (the 1/sqrt(Neff) attenuation cancels against the
output's own scale), so fp8's ~1.8% rms quantization alone costs
~2e-2.  The additive attention bias is applied MULTIPLICATIVELY:
the host ships eb = exp(bias) (0 where causally masked) in bf16 and
the device computes P = exp(qk) * eb -- exact masking for free.

Engine budget per core, measured (Act is the wall; exp is
scalar-engine-only at 0.833 ns/col + 185 ns SBUF-access per instr):
  Act  57.9us  exp (60 instrs) + 4 tail proj copies
  PE   54.1us  QK 21.8 + PV 21.8 + proj 10.2 + overheads
  DVE  ~48us   eb-mul share (2x bf16 mode) + reciprocal + normalize
               + PSUM->SBUF proj copies (384-wide)
  Pool ~42us   eb-mul share (chunks 2-3 only) + eb/va/out DMA (SWDGE)
  SP   ~44us   eb/qk/w/out DMA
Modeled span 69315 ns (vs 83772 baseline).

Scheduling (the engine queues are IN-ORDER, so emission order is the
schedule): three head streams round-robin per region (head-major for
the first/last chunk); PV lags 1-3 regions; psl is a 2-deep [128,1024]
PSUM ring; psy is one bank per head, allocated lazily at first use.
Chunk j's final-region PV flush + normalizations and its projection
blocks + out DMAs are deferred into chunk j+1's emission stream and
drained ONE per (r,h) iteration -- any burst of deferred work
head-of-line-blocks the in-order PE queue on a DVE copy and starves
Act.  Mid-stream projection PSUM ping-pongs on the single psp bank
(no psy-tag reuse -> no cross-chunk buffer-ordering deadlocks); only
the last chunk's burst rotates all 4 spare banks and splits its copies
Act/DVE, and its regions run DESCENDING per head so head-transition
psl round-trips hide under the wide low-r exps (first PV popped per
head must be full-width: a start=False matmul may not write a mix of
pending-zero and written PSUM bytes).  Startup DMAs are ordered by first-use deadline with the eb
stream mostly on SP; Pool's transfers all land before chunk 2 so
mid-stream Pool eb-muls never queue behind a DMA.
"""

import math

import numpy as np
import ml_dtypes

B, H, T, D = 2, 12, 2048, 64
DIM = H * D
NCORES = 8
HPC = 3            # heads per core
P = 128
QC = 512           # q-chunk width
NJ = T // QC       # 4 q-chunks
NT = T // P        # 16 s-tiles

BF = ml_dtypes.bfloat16

_PROGRAM = None


def _c0(i, j):
    return max(0, P * i - QC * j)


def _regions(j):
    """[(r, wa, wb)] for chunk j: region r covers s-tiles 2r, 2r+1."""
    out = []
    for r in range(2 * (j + 1)):
        wa = QC - _c0(2 * r, j)
        wb = QC - _c0(2 * r + 1, j)
        out.append((r, wa, wb))
    return out


# eb packed offsets, (j, h)-major: chunk j's blocks for all heads are
# contiguous; within (j, h) the region blocks are r-major, each block
# [128 part, wa+wb] bf16 (tile-a cols then tile-b cols, causally trimmed).
EB_JCOLS = [sum(wa + wb for _, wa, wb in _regions(_j)) for _j in range(NJ)]
EB_JOFF = [0]
for _j in range(NJ):
    EB_JOFF.append(EB_JOFF[-1] + HPC * EB_JCOLS[_j])
EB_TOT = EB_JOFF[-1]   # 52224 cols per core


def _eb_off(j, h):
    return EB_JOFF[j] + h * EB_JCOLS[j]


def _build_program():
    import concourse.bass as bass
    import concourse.mybir as mybir
    import concourse.tile as tile
    from concourse import bacc
    from contextlib import ExitStack

    dt = mybir.dt
    f32 = dt.float32
    bf16 = dt.bfloat16
    EXP = mybir.ActivationFunctionType.Exp
    ds = bass.ds

    nc = bacc.Bacc("TRN2", num_devices=NCORES)
    k16_d = nc.declare_dram_parameter("k16", [D, HPC * T], bf16, isOutput=False)
    q16_d = nc.declare_dram_parameter("q16", [D, HPC * T], bf16, isOutput=False)
    eb_d = nc.declare_dram_parameter("eb", [P, EB_TOT], bf16,
                                     isOutput=False)
    va_d = nc.declare_dram_parameter("va", [P, HPC * NT * P], bf16,
                                     isOutput=False)
    w01_d = nc.declare_dram_parameter("w01", [P, DIM], bf16, isOutput=False)
    w2_d = nc.declare_dram_parameter("w2", [D, DIM], bf16, isOutput=False)
    out_d = nc.declare_dram_parameter("out", [T, DIM], bf16, isOutput=True)

    OB = 384  # projection o-block width (two blocks per 768-wide tb)

    with tile.TileContext(nc) as tc, ExitStack() as ctx:
        const_pool = ctx.enter_context(tc.tile_pool(name="const", bufs=1))
        k16_t = const_pool.tile([D, HPC * T], bf16)
        q16_t = const_pool.tile([D, HPC * T], bf16)
        eb_t = const_pool.tile([P, EB_TOT], bf16)
        va_t = const_pool.tile([P, HPC * NT * P], bf16)
        w01_t = const_pool.tile([P, DIM], bf16)
        w2_t = const_pool.tile([D, DIM], bf16)

        def eb_dma(eng, h, j, r0, r1):
            regs = _regions(j)
            o0 = _eb_off(j, h) + sum(wa + wb for _, wa, wb in regs[:r0])
            sz = sum(wa + wb for _, wa, wb in regs[r0:r1])
            eng.dma_start(eb_t[:, ds(o0, sz)], eb_d[:, ds(o0, sz)])

        # ---- startup DMAs, ordered by first-use deadline.  SP carries
        # most of the eb stream (it has no compute); Pool's DMAs all land
        # before chunk-2 so mid-stream Pool eb-muls never queue behind a
        # transfer.  Chunk 0/1 muls run on DVE only for the same reason.
        nc.sync.dma_start(k16_t[:, ds(0, QC)], k16_d[:, ds(0, QC)])
        nc.sync.dma_start(q16_t[:, ds(0, QC)], q16_d[:, ds(0, QC)])
        eb_dma(nc.gpsimd, 0, 0, 0, 2)
        nc.sync.dma_start(k16_t[:, ds(T, QC)], k16_d[:, ds(T, QC)])
        nc.sync.dma_start(q16_t[:, ds(T, QC)], q16_d[:, ds(T, QC)])
        eb_dma(nc.gpsimd, 1, 0, 0, 2)
        nc.sync.dma_start(k16_t[:, ds(2 * T, QC)], k16_d[:, ds(2 * T, QC)])
        nc.sync.dma_start(q16_t[:, ds(2 * T, QC)], q16_d[:, ds(2 * T, QC)])
        eb_dma(nc.gpsimd, 2, 0, 0, 2)
        nc.gpsimd.dma_start(va_t[:, ds(0, NT * P)], va_d[:, ds(0, NT * P)])
        # chunk-1 q for all heads before the k tails (their first use is
        # earlier); k tails cover s-tiles 4..15 which only chunk-1 r2+ needs
        nc.sync.dma_start(q16_t[:, ds(QC, QC)], q16_d[:, ds(QC, QC)])
        nc.sync.dma_start(q16_t[:, ds(T + QC, QC)], q16_d[:, ds(T + QC, QC)])
        nc.sync.dma_start(q16_t[:, ds(2 * T + QC, QC)],
                          q16_d[:, ds(2 * T + QC, QC)])
        nc.sync.dma_start(w01_t[:], w01_d[:])
        nc.sync.dma_start(w2_t[:], w2_d[:])
        nc.sync.dma_start(k16_t[:, ds(QC, T - QC)], k16_d[:, ds(QC, T - QC)])
        eb_dma(nc.sync, 0, 1, 0, 2)
        nc.gpsimd.dma_start(va_t[:, ds(NT * P, NT * P)],
                            va_d[:, ds(NT * P, NT * P)])
        nc.sync.dma_start(k16_t[:, ds(T + QC, T - QC)],
                          k16_d[:, ds(T + QC, T - QC)])
        eb_dma(nc.gpsimd, 2, 1, 0, 4)
        eb_dma(nc.sync, 0, 1, 2, 4)
        nc.sync.dma_start(k16_t[:, ds(2 * T + QC, T - QC)],
                          k16_d[:, ds(2 * T + QC, T - QC)])
        eb_dma(nc.sync, 1, 1, 0, 4)
        nc.gpsimd.dma_start(va_t[:, ds(2 * NT * P, NT * P)],
                            va_d[:, ds(2 * NT * P, NT * P)])
        eb_dma(nc.gpsimd, 0, 2, 0, 3)
        eb_dma(nc.gpsimd, 2, 2, 0, 6)
        nc.sync.dma_start(q16_t[:, ds(2 * QC, QC)], q16_d[:, ds(2 * QC, QC)])
        nc.sync.dma_start(q16_t[:, ds(T + 2 * QC, QC)],
                          q16_d[:, ds(T + 2 * QC, QC)])
        nc.sync.dma_start(q16_t[:, ds(2 * T + 2 * QC, QC)],
                          q16_d[:, ds(2 * T + 2 * QC, QC)])
        eb_dma(nc.sync, 0, 2, 3, 6)
        eb_dma(nc.sync, 1, 2, 0, 6)
        nc.sync.dma_start(q16_t[:, ds(3 * QC, QC)], q16_d[:, ds(3 * QC, QC)])
        nc.sync.dma_start(q16_t[:, ds(T + 3 * QC, QC)],
                          q16_d[:, ds(T + 3 * QC, QC)])
        nc.sync.dma_start(q16_t[:, ds(2 * T + 3 * QC, QC)],
                          q16_d[:, ds(2 * T + 3 * QC, QC)])
        eb_dma(nc.gpsimd, 0, 3, 0, 4)
        eb_dma(nc.sync, 0, 3, 4, 8)
        eb_dma(nc.sync, 1, 3, 0, 8)
        eb_dma(nc.sync, 2, 3, 0, 8)

        def k_ap(h, i):
            return k16_t[:, ds(h * T + i * P, P)]

        def q_ap(h, j, c0):
            return q16_t[:, ds(h * T + j * QC + c0, QC - c0)]

        def va_ap(h, i):
            return va_t[:, ds((h * NT + i) * P, P)]

        with (
            tc.tile_pool(name="pexp", bufs=8) as pexp_pool,
            tc.tile_pool(name="rec", bufs=2) as rec_pool,
            tc.tile_pool(name="yt01", bufs=2) as yt01_pool,
            tc.tile_pool(name="yt2", bufs=2) as yt2_pool,
            tc.tile_pool(name="obig", bufs=4) as obig_pool,
            tc.tile_pool(name="psl", bufs=2, space="PSUM") as psl_pool,
            tc.tile_pool(name="psy", bufs=1, space="PSUM") as psy_pool,
            tc.tile_pool(name="psp", bufs=1, space="PSUM") as psp_pool,
        ):
            defer_q = []   # deferred thunks (proj blocks, out/eb DMAs)

            def drain(n):
                for _ in range(min(n, len(defer_q))):
                    defer_q.pop(0)()

            COPY = mybir.ActivationFunctionType.Copy

            def make_blk(tb, ob, bank, yt01_c, yt2_c, o_c, act_copy=False):
                # proj PSUM rotates over psp + the h2/h1 psy banks (their
                # next-chunk PVs are the last to need the banks back; lazy
                # psy allocation keeps buffer-reuse deps acyclic with the
                # in-order engine queues).  The last chunk also uses h0.
                def blk():
                    if bank == 0:
                        psp_t = psp_pool.tile([P, OB], f32)
                    else:
                        psp_t = psy_pool.tile([P, OB], f32, name="psy",
                                              tag=["h2", "h1", "h0"][bank - 1])
                    nc.tensor.matmul(
                        psp_t[:],
                        lhsT=yt01_c[:, tb * P:(tb + 1) * P],
                        rhs=w01_t[:, ds(ob * OB, OB)],
                        start=True, stop=False,
                    )
                    nc.tensor.matmul(
                        psp_t[:],
                        lhsT=yt2_c[:, tb * P:(tb + 1) * P],
                        rhs=w2_t[:, ds(ob * OB, OB)],
                        start=False, stop=True,
                    )
                    if act_copy:
                        nc.scalar.activation(
                            o_c[:, ds(tb * DIM + ob * OB, OB)], psp_t[:], COPY
                        )
                    else:
                        nc.vector.tensor_copy(
                            o_c[:, ds(tb * DIM + ob * OB, OB)], psp_t[:]
                        )
                return blk

            def make_out_dma(eng, jj, tb, o_c, ob=None):
                def dma():
                    if ob is None:
                        eng.dma_start(
                            out_d[jj * QC + tb * P:jj * QC + (tb + 1) * P, :],
                            o_c[:, ds(tb * DIM, DIM)],
                        )
                    else:
                        eng.dma_start(
                            out_d[jj * QC + tb * P:jj * QC + (tb + 1) * P,
                                  ob * OB:(ob + 1) * OB],
                            o_c[:, ds(tb * DIM + ob * OB, OB)],
                        )
                return dma

            mul_toggle = [0]

            for j in range(NJ):
                regs = _regions(j)
                nreg = len(regs)
                roffs = [0] * len(regs)
                for _r, _wa, _wb in regs[:-1]:
                    roffs[_r + 1] = roffs[_r] + _wa + _wb
                psy_ts = [None] * HPC
                pvqs = [[] for _ in range(HPC)]

                def get_psy(h, psy_ts=psy_ts):
                    if psy_ts[h] is None:
                        psy_ts[h] = psy_pool.tile([P, QC], f32, name="psy",
                                                  tag=f"h{h}")
                    return psy_ts[h]

                pv_done = [0] * HPC

                def emit_pv(h, r, pe_t, wa, wb, nreg=nreg,
                            get_psy=get_psy, pv_done=pv_done):
                    first = pv_done[h] == 0
                    pv_done[h] += 1
                    final = pv_done[h] == nreg
                    for t in range(2):
                        i = 2 * r + t
                        pos = 0 if t == 0 else wa
                        w = wa if t == 0 else wb
                        nc.tensor.matmul(
                            get_psy(h)[:, QC - w:QC],
                            lhsT=va_ap(h, i),
                            rhs=pe_t[:, ds(pos, w)],
                            start=(first and t == 0),
                            stop=(final and t == 1),
                        )

                if j == NJ - 1:
                    # heads 0/1 descending: head-transition psl round-trips
                    # hide under wide exps; the last head ascending so the
                    # post-last-exp tail chain hangs off a small region
                    iter_order = []
                    for hh in range(HPC):
                        _o = regs if hh == HPC - 1 else list(reversed(regs))
                        iter_order += [(r, wa, wb, hh) for r, wa, wb in _o]
                elif j == 0:
                    iter_order = [(r, wa, wb, h) for h in range(HPC)
                                  for r, wa, wb in regs]
                else:
                    iter_order = [(r, wa, wb, h) for r, wa, wb in regs
                                  for h in range(HPC)]
                yt01_t = yt01_pool.tile([P, QC], bf16)
                yt2_t = yt2_pool.tile([D, QC], bf16)

                def make_tailnorm(h, yt01_c, yt2_c, pvq, psy_c, epv,
                                  split=False):
                    def tail():
                        while pvq:
                            epv(*pvq.pop(0))
                        rec_t = rec_pool.tile([D, QC], f32)
                        nc.vector.reciprocal(rec_t[:], psy_c(h)[D:2 * D, :])
                        if h == 0:
                            ydst = yt01_c[0:D, :]
                        elif h == 1:
                            ydst = yt01_c[D:2 * D, :]
                        else:
                            ydst = yt2_c[:]
                        if split:
                            for tb in range(4):
                                nc.vector.tensor_mul(
                                    ydst[:, tb * P:(tb + 1) * P],
                                    psy_c(h)[0:D, tb * P:(tb + 1) * P],
                                    rec_t[:, tb * P:(tb + 1) * P])
                        else:
                            nc.vector.tensor_mul(ydst, psy_c(h)[0:D, :],
                                                 rec_t[:])
                    return tail

                it = 0
                last = j == NJ - 1
                for r, wa, wb, h in iter_order:
                    if True:
                        po = wa + wb
                        psl_t = psl_pool.tile([P, 2 * QC], f32)
                        pe_t = pexp_pool.tile([P, 2 * QC], bf16)
                        nc.tensor.matmul(
                            psl_t[:, 0:wa],
                            lhsT=k_ap(h, 2 * r),
                            rhs=q_ap(h, j, QC - wa),
                            start=True, stop=True,
                        )
                        nc.tensor.matmul(
                            psl_t[:, wa:po],
                            lhsT=k_ap(h, 2 * r + 1),
                            rhs=q_ap(h, j, QC - wb),
                            start=True, stop=True,
                        )
                        nc.scalar.activation(
                            pe_t[:, 0:po], psl_t[:, 0:po], EXP
                        )
                        if j < 2:
                            meng = nc.vector
                        else:
                            meng = (nc.gpsimd if mul_toggle[0] % 5 < 3
                                    else nc.vector)
                            mul_toggle[0] += 1
                        meng.tensor_mul(
                            pe_t[:, 0:po],
                            pe_t[:, 0:po],
                            eb_t[:, ds(_eb_off(j, h) + roffs[r], po)],
                        )
                        pvqs[h].append((h, r, pe_t, wa, wb))
                        lag = 1 if j == 0 else (3 if last else 2)
                        hpos = it % nreg if last else r

                        def pop_pv(h=h):
                            # first PV per head must write the whole psy
                            # bank (a start=False matmul may not touch a
                            # mix of pending-zero and written bytes)
                            if pv_done[h] == 0:
                                for ii, e in enumerate(pvqs[h]):
                                    if e[3] == QC:
                                        return pvqs[h].pop(ii)
                            return pvqs[h].pop(0)

                        if last and hpos >= nreg - 2:
                            while pvqs[h]:
                                emit_pv(*pop_pv())
                        elif len(pvqs[h]) > (lag if it < 6 else 1):
                            emit_pv(*pop_pv())
                        if last and hpos == nreg - 1:
                            # this head's stream is done; flush + norm now
                            # so only h2's norm remains after the last exp
                            defer_q.append(make_tailnorm(
                                h, yt01_t, yt2_t, pvqs[h], get_psy, emit_pv,
                                split=False))
                        if it >= 2:
                            drain(1)
                        it += 1

                if not last:
                    # tails go to the queue FRONT: they must drain before
                    # the next chunk's first lazy-psy PV allocation
                    for h in reversed(range(HPC)):
                        defer_q.insert(0, make_tailnorm(
                            h, yt01_t, yt2_t, pvqs[h], get_psy, emit_pv))

                o_t = obig_pool.tile([P, 4 * DIM], bf16)
                for tb in range(4):
                    for ob in range(2):
                        # mid-stream proj lives on psp only (no psy-tag
                        # ordering constraints); the last chunk's burst
                        # rotates all 4 banks and splits copies Act/DVE
                        defer_q.append(make_blk(
                            tb, ob, (tb * 2 + ob) % 4 if last else 0,
                            yt01_t, yt2_t, o_t,
                            act_copy=(last and (tb * 2 + ob) % 2 == 1)))
                        if last:
                            # per-half DMAs right after their copy so the
                            # final transfer is small and starts early
                            defer_q.append(make_out_dma(
                                nc.gpsimd if ob == 0 else nc.sync,
                                j, tb, o_t, ob=ob))
                    if not last:
                        defer_q.append(make_out_dma(
                            nc.sync if tb % 2 == 0 else nc.gpsimd,
                            j, tb, o_t))

            drain(len(defer_q))

    nc.finalize()
    return nc


def _get_program():
    global _PROGRAM
    if _PROGRAM is None:
        _PROGRAM = _build_program()
    return _PROGRAM


def make_in_maps(q, k, v, attn_bias, W_proj):
    """Host-side sharding/layout prep: one input map per core."""
    q = np.asarray(q, dtype=np.float32)
    k = np.asarray(k, dtype=np.float32)
    v = np.asarray(v, dtype=np.float32)
    attn_bias = np.asarray(attn_bias, dtype=np.float32)
    W_proj = np.asarray(W_proj, dtype=np.float32)

    scale = 1.0 / math.sqrt(D)
    w_heads = W_proj.reshape(H, D, DIM)
    smask = np.arange(T)[:, None] <= np.arange(T)[None, :]  # [s, q] valid

    in_maps = []
    for c in range(NCORES):
        bb = c // 4
        h0 = HPC * (c % 4)
        k16 = np.empty((HPC, D, T), dtype=BF)
        q16 = np.empty((HPC, D, T), dtype=BF)
        eb = np.empty((P, EB_TOT), dtype=BF)
        va = np.zeros((P, HPC, NT, P), dtype=np.float32)
        for h in range(HPC):
            hh = h0 + h
            k16[h] = k[bb, hh].T.astype(BF)
            q16[h] = (q[bb, hh].T * scale).astype(BF)
            ebf = np.exp(attn_bias[bb, hh].T) * smask  # [s, q]
            for j in range(NJ):
                off = _eb_off(j, h)
                for r, wa, wb in _regions(j):
                    eb[:, off:off + wa] = ebf[
                        2 * r * P:(2 * r + 1) * P,
                        (j + 1) * QC - wa:(j + 1) * QC].astype(BF)
                    off += wa
                    eb[:, off:off + wb] = ebf[
                        (2 * r + 1) * P:(2 * r + 2) * P,
                        (j + 1) * QC - wb:(j + 1) * QC].astype(BF)
                    off += wb
            va[:, h, :, 0:D] = (
                v[bb, hh].reshape(NT, P, D).transpose(1, 0, 2))
            va[:, h, :, D:] = 1.0
        # k16/q16: [d, (h, t)] with heads contiguous per partition
        in_maps.append(
            {
                "k16": k16.transpose(1, 0, 2).reshape(D, HPC * T),
                "q16": q16.transpose(1, 0, 2).reshape(D, HPC * T),
                "eb": eb,
                "va": va.reshape(P, HPC * NT * P).astype(BF),
                "w01": np.ascontiguousarray(
                    w_heads[h0:h0 + 2].reshape(P, DIM)).astype(BF),
                "w2": w_heads[h0 + 2].astype(BF),
                "out": np.zeros((T, DIM), dtype=BF),
            }
        )
    return in_maps


def assemble_output(results):
    """Sum the 4 per-core partial projections for each batch."""
    out = np.zeros((B, T, DIM), dtype=np.float32)
    for c in range(NCORES):
        out[c // 4] += np.asarray(results[c]["out"], dtype=np.float32)
    return out


def kernel(q, k, v, attn_bias, W_proj):
    from concourse.bass_utils import run_bass_kernel_spmd

    nc = _get_program()
    in_maps = make_in_maps(q, k, v, attn_bias, W_proj)
    res = run_bass_kernel_spmd(nc, in_maps, list(range(NCORES)))
    return assemble_output(res.results)
